# revision 4
# baseline (speedup 1.0000x reference)
"""Trainium2 Bass kernel v2 for the DisLoss (segment-reduce) problem.

Same math as the baseline (see kernel.py docstring), restructured:
  - inputs host-cast to bf16 (rel-err budget 2e-2; bf16 costs ~1e-4):
    halves the launch-A DMA stream and removes all on-device casts.
  - pc part split across engines: f1 via the fused custom DVE sqdiff
    (1x), f2 via DVE subtract (bf16 2x) + ACT Square with accum_out
    (fused square + row-sum).
  - center sums accumulated into held [64, 1024] PSUM tiles via
    per-tile one-hot weights (oh_t[p, g] = g == 16t + p//8), so no
    partition-offset tricks and only 2 PSUM->SBUF copies at the end.
  - ||center||^2 (raw scale) computed in launch A on ACT (square +
    accum along the free axis = the d axis in row-major layout) and
    exported; launch B loses its whole sq pipeline.
  - launch B: host pre-scales local centers by -2 (exact, power of 2)
    so P' = -2*Gram + sq_h after the augmented fold, and the final
    math is one scalar_tensor_tensor (P' + sq_g) * invm and one ACT
    Sqrt(u/256) with accum_out = row-sum = dist_an * (G-1).
"""

import numpy as np
import ml_dtypes

import concourse.bacc as bacc
import concourse.mybir as mybir
import concourse.tile as tile
from concourse.bass_utils import run_bass_kernel_spmd

import concourse.dve_ops as dve_ops
from concourse.dve_ops import DveOp, _ref_body_sum
from concourse.dve_spec import Spec, Src0, Src1, Zero, lower, sq
from concourse.dve_uop import DveOpSpec
from operator import add

_NAME = "SQDIFF_ACC_ANT"


def _make_spec():
    return Spec(
        body=sq(Src0 - Src1),
        accum=add,
        accum_init=Zero,
        reference=_ref_body_sum(
            lambda in0, in1, c0, c1, c2: (in0.astype(np.float32) - in1) ** 2
        ),
    )


def register():
    for op in dve_ops.OPS:
        if op.name == _NAME:
            return op
    row = dve_ops._CUSTOM_DVE_ROW_BASE + len(dve_ops.OPS)
    assert row < 0x20
    spec = _make_spec()
    shas = {}
    for ver in ("v3", "v4"):
        lowered = DveOpSpec(name=_NAME, opcode=row, uops=lower(spec, ver=ver),
                            rd1_en=True)
        shas[ver] = lowered.sha(ver)
    op = DveOp(_NAME, spec, subdim=False, uops_sha=shas)
    dve_ops.OPS.append(op)
    dve_ops._SUB_OPCODE_FOR_NAME[_NAME] = row
    dve_ops.CUSTOM_DVE_SPECS[_NAME] = spec
    return op


SQDIFF = register()


def sqdiff_acc(nc, out, accum_out, in0, in1):
    return nc.vector._custom_dve(
        SQDIFF, out=out, in0=in0, in1=in1, accum_out=accum_out
    )


# light tile tail: drain + sem-only barrier (see baseline kernel.py)
import contextlib

from concourse.vector_clock import ScopedClock


def _light_drain_and_barrier(self, tick_clock, wait_clock):
    drain_inst = self.nc.sync.drain()
    wait_clock.add_sem_waits(
        drain_inst.ins, ScopedClock({None: tick_clock.global_clock})
    )
    self.nc.all_engine_barrier(sem_only=True)
    popped = self.nc._tile_sem_poison_stack.pop()
    assert popped is self._sem_poison


@contextlib.contextmanager
def _light_tile_tail():
    orig = tile.TileContext._drain_and_barrier
    tile.TileContext._drain_and_barrier = _light_drain_and_barrier
    try:
        yield
    finally:
        tile.TileContext._drain_and_barrier = orig


NC = 8
B = 4096
D = 2048
K = 8
G = B // K          # 512 groups
RPC = B // NC       # 512 rows per core per chunk
GPC = G // NC       # 64 groups per core
NT = RPC // 128     # 4 row tiles per chunk per core
GPT = 128 // K      # 16 groups per 128-row tile
NJ = 2              # 2 column chunks of 1024
JW = D // NJ        # 1024

F32 = mybir.dt.float32
BF16 = mybir.dt.bfloat16
AX = mybir.AxisListType
ALU = mybir.AluOpType
ACTF = mybir.ActivationFunctionType
BF = ml_dtypes.bfloat16


def _build_launch_a():
    nc = bacc.Bacc(
        "TRN2",
        target_bir_lowering=False,
        debug=False,
        enable_asserts=False,
        num_devices=NC,
    )
    x1 = nc.dram_tensor("x1", [RPC, D], BF16, kind="ExternalInput").ap()
    x2 = nc.dram_tensor("x2", [RPC, D], BF16, kind="ExternalInput").ap()
    xm = nc.dram_tensor("xm", [RPC, D], BF16, kind="ExternalInput").ap()
    # mavg[q, p] = (q//K == p//K) / K   block-diag row-averager [128, 128]
    mv_in = nc.dram_tensor("mavg", [128, 128], BF16, kind="ExternalInput").ap()
    # oh4[p, NT*?]: oh_t[p, g] = (g == GPT*t + p//K), packed [128, NT*GPC]
    oh_in = nc.dram_tensor("oh4", [128, NT * GPC], BF16, kind="ExternalInput").ap()
    ones_in = nc.dram_tensor("ones128", [128, 1], BF16, kind="ExternalInput").ap()
    # csums widened: col D holds sq (raw ||csum||^2) as bf16
    cs_out = nc.dram_tensor("csums", [GPC, D + 8], BF16, kind="ExternalOutput").ap()
    misc_out = nc.dram_tensor("misc", [1, 8], F32, kind="ExternalOutput").ap()

    with tile.TileContext(nc) as tc:
        with (
            tc.tile_pool(name="consts", bufs=1) as consts,
            tc.tile_pool(name="xf", bufs=2 * NT) as xf,
            tc.tile_pool(name="xm_p", bufs=NT) as xm_p,
            tc.tile_pool(name="scr", bufs=2) as scr,
            tc.tile_pool(name="acc", bufs=1) as acc,
            tc.tile_pool(name="fin", bufs=1) as fin,
            tc.tile_pool(name="ps_cmb", bufs=2, space="PSUM") as ps_cmb,
            tc.tile_pool(name="ps_ct", bufs=1, space="PSUM") as ps_ct,
        ):
            # consts on the scalar sequencer: keeps the sync queue's
            # descgen + FIFO stream clear for xm_0/x1_0
            mv = consts.tile([128, 128], BF16)
            oh4 = consts.tile([128, NT * GPC], BF16)
            ones128 = consts.tile([128, 1], BF16)
            nc.scalar.dma_start(mv[:], mv_in[:])
            nc.scalar.dma_start(oh4[:], oh_in[:])
            nc.scalar.dma_start(ones128[:], ones_in[:])

            # preload both ACT table families while DMA streams
            dmy = consts.tile([128, 2], F32)
            nc.scalar.activation(dmy[:, 0:1], mv[:, 0:1], ACTF.Square)
            nc.scalar.activation(dmy[:, 1:2], mv[:, 0:1], ACTF.Sqrt)

            # input loads up front on ONE queue in consumption (trio)
            # order -- concurrent queues round-robin at packet
            # granularity and stretch early-tile completion 3x.
            xm_ts, x1_ts, x2_ts = [], [], []
            for t in range(NT):
                xm_t = xm_p.tile([128, D], BF16, tag="xm")
                x1_t = xf.tile([128, D], BF16, tag="x1")
                x2_t = xf.tile([128, D], BF16, tag="x2")
                nc.sync.dma_start(xm_t[:], xm[t * 128 : (t + 1) * 128, :])
                nc.sync.dma_start(x1_t[:], x1[t * 128 : (t + 1) * 128, :])
                nc.sync.dma_start(x2_t[:], x2[t * 128 : (t + 1) * 128, :])
                xm_ts.append(xm_t)
                x1_ts.append(x1_t)
                x2_ts.append(x2_t)

            # held center-sum accumulators (raw sums over f1,f2 rows)
            ct_ps = [ps_ct.tile([GPC, JW], F32, name=f"ct_ps{j}")
                     for j in range(NJ)]

            # dsq accumulator columns: [f1: NT*NJ][f2: NT*NJ]
            dsq = acc.tile([128, 2 * NT * NJ], F32)

            def emit_ct(t, j):
                oh_t = oh4[:, t * GPC : (t + 1) * GPC]
                jl = JW * j
                for h in range(2):
                    hl, hh = 512 * h, 512 * (h + 1)
                    nc.tensor.matmul(ct_ps[j][:, hl:hh], oh_t,
                                     x1_ts[t][:, jl + hl : jl + hh],
                                     start=(t == 0), stop=False)
                    nc.tensor.matmul(ct_ps[j][:, hl:hh], oh_t,
                                     x2_ts[t][:, jl + hl : jl + hh],
                                     start=False, stop=(t == NT - 1))

            for t in range(NT):
                xm_t, x1_t, x2_t = xm_ts[t], x1_ts[t], x2_ts[t]
                if t == NT - 1:
                    # last tile: finish the ct accumulation FIRST so the
                    # center export (ACT) overlaps the remaining DVE
                    # sqdiffs instead of trailing them
                    for j in range(NJ):
                        emit_ct(t, j)
                # both cmb tiles first: the x2-gated ct matmuls must not
                # sit between them in the PE FIFO (starves the DVE)
                cmbs = []
                for j in range(NJ):
                    jl = JW * j
                    cmb = ps_cmb.tile([128, JW], F32, tag="cmb")
                    for h in range(2):  # ISA caps matmul moving at 512 cols
                        hl, hh = 512 * h, 512 * (h + 1)
                        nc.tensor.matmul(cmb[:, hl:hh], mv[:],
                                         xm_t[:, jl + hl : jl + hh],
                                         start=True, stop=True)
                    cmbs.append(cmb)
                for j in range(NJ):
                    jl, jh = JW * j, JW * (j + 1)
                    # both chunks: fused sqdiff on DVE, reading cmb PSUM
                    o1 = scr.tile([128, JW], BF16, tag="o1")
                    c = NJ * t + j
                    sqdiff_acc(nc, o1[:], dsq[:, c : c + 1],
                               x1_t[:, jl:jh], cmbs[j][:])
                    o2 = scr.tile([128, JW], BF16, tag="o2")
                    c2 = NT * NJ + c
                    sqdiff_acc(nc, o2[:], dsq[:, c2 : c2 + 1],
                               x2_t[:, jl:jh], cmbs[j][:])
                if t < NT - 1:
                    for j in range(NJ):
                        emit_ct(t, j)

            # center sums -> SBUF bf16 + squares, all on ACT (idle while
            # the DVE sqdiff stream finishes)
            ct_sb = fin.tile([GPC, D + 8], BF16)
            sqp = fin.tile([GPC, NJ], F32)
            for j in range(NJ):
                nc.scalar.copy(ct_sb[:, j * JW : (j + 1) * JW], ct_ps[j][:])
                ct2 = scr.tile([128, JW], BF16, tag="o2" if j else "o1")
                nc.scalar.activation(ct2[:GPC, :],
                                     ct_sb[:, j * JW : (j + 1) * JW],
                                     ACTF.Square, accum_out=sqp[:, j : j + 1])
            # sq partials ride in csums cols D, D+1 (bf16, host adds
            # them) -- keeps the export chain off the busy DVE
            nc.scalar.copy(ct_sb[:, D : D + 2], sqp[:])
            nc.gpsimd.memset(ct_sb[:, D + 2 : D + 8], 0)
            nc.sync.dma_start(cs_out[:], ct_sb[:])

            # pc = sqrt(sum_j dsq) [128, 8] -> ones-matmul -> [1, 8]
            pc2 = acc.tile([128, 2 * NT], F32)
            dv = dsq[:].rearrange("p (c j) -> p c j", j=NJ)
            nc.vector.reduce_sum(pc2[:], dv, axis=AX.X)
            pc_sb = acc.tile([128, 2 * NT], BF16)
            nc.scalar.activation(pc_sb[:], pc2[:], ACTF.Sqrt)
            ps_pc = ps_cmb.tile([1, 2 * NT], F32, tag="cmb")
            nc.tensor.matmul(ps_pc[:], ones128[:], pc_sb[:],
                             start=True, stop=True)
            misc = fin.tile([1, 8], F32)
            nc.scalar.copy(misc[:], ps_pc[:])
            nc.sync.dma_start(misc_out[:], misc[:])

    nc.compile()
    return nc


def _build_launch_b():
    nc = bacc.Bacc(
        "TRN2",
        target_bir_lowering=False,
        debug=False,
        enable_asserts=False,
        num_devices=NC,
    )
    KT = D // 128  # 16 k-tiles over the feature dim
    ct_in = nc.dram_tensor("ctp", [128, KT * G], BF16, kind="ExternalInput").ap()
    # local centers pre-scaled by -2 on host (exact)
    cl_in = nc.dram_tensor("clp2", [128, KT * GPC], BF16, kind="ExternalInput").ap()
    # packed small consts, one load each: pk = invm | sql | o64 (fp32),
    # sv = sqv | ones1 (bf16 row)
    pk_in = nc.dram_tensor("pk", [GPC, G + 2], F32, kind="ExternalInput").ap()
    sv_in = nc.dram_tensor("sv", [1, G + GPC], BF16, kind="ExternalInput").ap()
    an_out = nc.dram_tensor("an", [1, 1], F32, kind="ExternalOutput").ap()

    with tile.TileContext(nc) as tc:
        with (
            tc.tile_pool(name="consts", bufs=1) as consts,
            tc.tile_pool(name="fin", bufs=1) as fin,
            tc.tile_pool(name="ps_g", bufs=1, space="PSUM") as ps_g,
        ):
            clp2 = consts.tile([128, KT * GPC], BF16)
            pk = consts.tile([GPC, G + 2], F32)
            sv = consts.tile([1, G + GPC], BF16)
            ctp = consts.tile([128, KT * G], BF16)
            nc.sync.dma_start(clp2[:], cl_in[:])
            nc.scalar.dma_start(pk[:], pk_in[:])
            nc.scalar.dma_start(sv[:], sv_in[:])
            invm = pk[:, 0:G]
            sql = pk[:, G : G + 1]
            o64 = pk[:, G + 1 : G + 2]
            sqv = sv[:, 0:G]
            ones1 = sv[:, G : G + GPC]
            # preload the Sqrt table while ctp streams
            dmy = consts.tile([GPC, 1], F32)
            nc.scalar.activation(dmy[:], sql, ACTF.Sqrt)
            # ctp via the otherwise-idle gpsimd SWDGE sequencer,
            # k-tile pairs so Gram can start on the first chunk
            QW = KT * G // 8
            for m in range(8):
                nc.gpsimd.dma_start(ctp[:, QW * m : QW * (m + 1)],
                                    ct_in[:, QW * m : QW * (m + 1)])

            # P' = -2*Gram + sq_h  (clp2 already scaled by -2; fold adds sq_h)
            P = ps_g.tile([GPC, G], F32)
            for k in range(KT):
                nc.tensor.matmul(P[:], clp2[:, GPC * k : GPC * (k + 1)],
                                 ctp[:, G * k : G * (k + 1)],
                                 start=(k == 0), stop=False)
            nc.tensor.matmul(P[:], ones1, sqv, start=False, stop=True)

            # u = (P' + sq_g) * invm ; an = sum_h sqrt(u / 256)
            u = fin.tile([GPC, G], F32)
            nc.vector.scalar_tensor_tensor(u[:], P[:], sql, invm,
                                           op0=ALU.add, op1=ALU.mult)
            dist = fin.tile([GPC, G], F32)
            an_sb = fin.tile([GPC, 1], F32)
            nc.scalar.activation(dist[:], u[:], ACTF.Sqrt, scale=1.0 / 256.0,
                                 accum_out=an_sb[:])
            # reduce [64,1] -> [1,1] on PE so the store is one descriptor
            ps_an = ps_g.tile([1, 1], F32)
            nc.tensor.matmul(ps_an[:], an_sb[:], o64, start=True, stop=True)
            anf = fin.tile([1, 1], F32)
            nc.scalar.copy(anf[:], ps_an[:])
            nc.sync.dma_start(an_out[:], anf[:])

    nc.compile()
    return nc


_CACHE = {}


def _get_kernels():
    if "a" not in _CACHE:
        with _light_tile_tail():
            _CACHE["a"] = _build_launch_a()
            _CACHE["b"] = _build_launch_b()
    return _CACHE["a"], _CACHE["b"]


def _consts_a():
    p = np.arange(128)
    mv = (p[:, None] // K == p[None, :] // K).astype(np.float32) / K
    oh4 = np.zeros((128, NT * GPC), np.float32)
    for t in range(NT):
        g = GPT * t + p // K
        oh4[p, t * GPC + g] = 1.0
    return mv.astype(BF), oh4.astype(BF)


def _validate(inputs, targets, k_size):
    assert inputs.shape == (3 * B, D), inputs.shape
    assert int(k_size) == K
    lab = np.asarray(targets).reshape(3, B)
    assert (lab == lab[0]).all(), "label layout must repeat per chunk"
    l0 = lab[0]
    assert (l0 == np.repeat(l0[::K], K)).all(), "labels must be contiguous k-blocks"
    blocks = l0[::K]
    assert len(np.unique(blocks)) == G, "group ids must be distinct"


def kernel(inputs, targets, k_size):
    inputs = np.asarray(inputs, dtype=np.float32)
    targets = np.asarray(targets)
    _validate(inputs, targets, k_size)

    nc_a, nc_b = _get_kernels()
    mv, oh4 = _consts_a()

    xb = inputs.astype(BF)  # host cast, round-to-nearest-even
    f1, f2, fm = xb[:B], xb[B : 2 * B], xb[2 * B :]
    in_maps_a = []
    for c in range(NC):
        sl = slice(c * RPC, (c + 1) * RPC)
        in_maps_a.append(
            {
                "x1": np.ascontiguousarray(f1[sl]),
                "x2": np.ascontiguousarray(f2[sl]),
                "xm": np.ascontiguousarray(fm[sl]),
                "mavg": mv,
                "oh4": oh4,
                "ones128": np.ones((128, 1), BF),
            }
        )
    res_a = run_bass_kernel_spmd(nc_a, in_maps_a, core_ids=list(range(NC)))

    # host glue: layout only
    cs_all = np.concatenate(
        [res_a.results[c]["csums"] for c in range(NC)], axis=0
    )  # [G, D+8] bf16: raw center sums | sq | pad
    ct = cs_all[:, :D].T  # [D, G]
    sq_all = (cs_all[:, D].astype(np.float32)
              + cs_all[:, D + 1].astype(np.float32))  # [G] raw ||csum||^2
    KT = D // 128
    ctp = np.ascontiguousarray(
        ct.reshape(KT, 128, G).transpose(1, 0, 2).reshape(128, KT * G))
    sv = np.concatenate(
        [sq_all, np.ones(GPC, np.float32)])[None, :].astype(BF)
    in_maps_b = []
    for c in range(NC):
        cl = ct[:, GPC * c : GPC * (c + 1)].astype(np.float32) * -2.0
        clp2 = np.ascontiguousarray(
            cl.astype(BF).reshape(KT, 128, GPC).transpose(1, 0, 2)
            .reshape(128, KT * GPC))
        pk = np.ones((GPC, G + 2), np.float32)
        pk[np.arange(GPC), GPC * c + np.arange(GPC)] = 0.0
        pk[:, G] = sq_all[GPC * c : GPC * (c + 1)]
        in_maps_b.append(
            {
                "ctp": ctp,
                "clp2": clp2,
                "pk": pk,
                "sv": sv,
            }
        )
    res_b = run_bass_kernel_spmd(nc_b, in_maps_b, core_ids=list(range(NC)))

    pc_sum = np.float64(0.0)
    for c in range(NC):
        pc_sum += res_a.results[c]["misc"].astype(np.float64).sum()
    an_sum = np.float64(0.0)
    for c in range(NC):
        an_sum += res_b.results[c]["an"].astype(np.float64).sum()
    num = pc_sum / B
    den = an_sum / (G - 1) / G
    return np.array(num / den, dtype=np.float32)
